# Initial kernel scaffold
#
"""Trainium2 Bass kernel: BN(eval) -> sign -> Conv1d(K=7,pad=3) -> alpha -> PReLU -> MaxPool2.

Strategy (hardcoded for B=64, CIN=64, L=4096, COUT=128, K=7):
  - Data-parallel over batch: 8 samples per NeuronCore x 8 cores; no
    cross-core communication.
  - Host folds BN into per-channel (scale, bias) and alpha into the conv
    weights (bf16); per-channel PReLU slope rides in as an SBUF vector.
  - A PAIR of samples shares one [128, L+8] bf16 "sign" tile: rows 0-63 =
    sample A, rows 64-127 = sample B (one chunked ScalarE Sign activation
    per input-DMA chunk so matmuls start early).
  - Conv = 7 PSUM-accumulated K=64 bf16 matmuls per 512-col tile; sample
    A's matmuls run on PE row-group 0-1 and B's on row-group 2-3
    concurrently (weights duplicated into both halves of the array), which
    measures ~94% of bf16 peak. A 30-matmul warmup flips the HAM clock
    gate to 8/8 before real work, and a dummy activation hoists the ACT
    table load to kernel start.
  - MaxPool(2) straight out of PSUM via DVE tensor_reduce(max) on
    [128, 256, 2] views; PReLU applied AFTER pooling (they commute) via
    the native ScalarE Prelu activation on bf16 halves, DMA'd out as bf16
    and widened to fp32 on the host.
  - Walrus in this toolchain accepts only one sync-wait per instruction,
    so the Tile-scheduled BIR is post-processed: multi-wait sync_info
    lists become single-wait EventSemaphore instructions (see
    _split_sync_waits_json).
"""

import json
import sys

for _p in ("/opt/trn_rl_repo", "/root/.axon_site/_ro/trn_rl_repo"):
    if _p not in sys.path:
        sys.path.append(_p)

import numpy as np
import ml_dtypes

import concourse.bass as bass
import concourse.tile as tile
from concourse import mybir
from concourse.bass_utils import run_bass_kernel_spmd

B, CIN, L, COUT, K = 64, 64, 4096, 128, 7
PAD = 3
BN_EPS = 1e-5
N_CORES = 8
BPC = B // N_CORES  # samples per core
LOUT = L // 2       # 2048 pooled length
NT = L // 512       # 8 output tiles of 512 cols

_CACHE: dict = {}


def build_program(use_act_prelu: bool = True) -> "bass.Bass":
    nc = bass.Bass(trn_type="TRN2")
    I8 = nc.dram_tensor("I8", [BPC, CIN, L], mybir.dt.float32, kind="ExternalInput")
    W = nc.dram_tensor("W", [128, K * 128], mybir.dt.bfloat16, kind="ExternalInput")
    SBp = nc.dram_tensor("SBp", [128, 4], mybir.dt.float32, kind="ExternalInput")
    O8 = nc.dram_tensor("O8", [BPC, COUT, LOUT], mybir.dt.bfloat16, kind="ExternalOutput")

    iflat = I8.ap().flatten_outer_dims()  # [BPC*64, 4096]
    oflat = O8.ap().flatten_outer_dims()  # [BPC*128, 2048]

    AF = mybir.ActivationFunctionType
    SGW = L + 8  # sg width: cols 0-2 zero pad, 3..L+2 data, L+3.. zero
    NHALF = NT // 2  # 4 l-tiles per half (A half + B half = 8 PSUM banks)
    with tile.TileContext(nc) as tc:
        with (
            tc.tile_pool(name="consts", bufs=1) as consts,
            tc.tile_pool(name="ipair", bufs=10) as ipool,
            tc.tile_pool(name="sgn", bufs=2) as spool,
            tc.tile_pool(name="pooled", bufs=2) as plpool,
            tc.tile_pool(name="outp", bufs=4) as opool,
            tc.tile_pool(name="ps", bufs=8, space="PSUM") as pspool,
        ):
            w_sb = consts.tile([128, K * 128], mybir.dt.bfloat16)
            nc.scalar.dma_start(w_sb[:], W.ap()[:])
            sb_sb = consts.tile([128, 4], mybir.dt.float32)
            nc.scalar.dma_start(sb_sb[:], SBp.ap()[:])
            # dummy activation: hoists the ACT table load to kernel start so
            # the first real Sign doesn't pay the ~1.3us table fetch later
            dummy = consts.tile([128, 4], mybir.dt.float32)
            nc.scalar.activation(
                dummy[:], sb_sb[:], mybir.ActivationFunctionType.Sign
            )
            # PE warmup while the first input chunks stream in: ~30 tiny
            # matmuls on the weight tile flip the HAM clock gate to 8/8
            warm = pspool.tile([128, 512], mybir.dt.float32, name="warm", tag="psb")
            for _ in range(30):
                nc.tensor.matmul(
                    warm[:, 0:64], w_sb[0:64, 0:128], w_sb[0:64, 0:64],
                    start=True, stop=True,
                )
            sgn_scale = sb_sb[:, 0:1]
            sgn_bias = sb_sb[:, 1:2]
            slope = sb_sb[:, 3:4]  # a

            NCHUNK = 8
            CW = L // NCHUNK
            HALO = K - 1  # so each 512-col matmul tile depends on ONE chunk
            for t in range(BPC // 2):
                # chunked input DMA + sign so the first matmuls start early;
                # chunks carry a 6-col halo (re-signed twice, same values)
                sg = spool.tile([128, SGW], mybir.dt.bfloat16)
                nc.gpsimd.memset(sg[:, 0:3], 0.0)
                nc.gpsimd.memset(sg[:, L + 3 : SGW], 0.0)
                for c in range(NCHUNK):
                    w = min(CW + HALO, L - CW * c)
                    ipc = ipool.tile([128, CW + HALO], mybir.dt.float32, name="ipc", tag="ipc")
                    nc.sync.dma_start(
                        ipc[:, 0:w],
                        iflat[128 * t : 128 * (t + 1), CW * c : CW * c + w],
                    )
                    nc.scalar.activation(
                        sg[:, 3 + CW * c : 3 + CW * c + w],
                        ipc[:, 0:w],
                        AF.Sign, bias=sgn_bias, scale=sgn_scale,
                    )

                pla = plpool.tile([128, LOUT], mybir.dt.bfloat16, name="pla", tag="pla")
                plb = plpool.tile([128, LOUT], mybir.dt.bfloat16, name="plb", tag="plb")
                for it in range(NT):
                    psa = pspool.tile([128, 512], mybir.dt.float32, name="psa", tag="psb")
                    psb = pspool.tile([128, 512], mybir.dt.float32, name="psb", tag="psb")
                    for k in range(K):
                        c0 = 512 * it + k
                        nc.tensor.matmul(
                            psa[:], w_sb[0:64, 128 * k : 128 * (k + 1)],
                            sg[0:64, c0 : c0 + 512],
                            start=(k == 0), stop=(k == K - 1),
                        )
                        nc.tensor.matmul(
                            psb[:], w_sb[64:128, 128 * k : 128 * (k + 1)],
                            sg[64:128, c0 : c0 + 512],
                            start=(k == 0), stop=(k == K - 1),
                        )
                    o0 = 256 * it
                    nc.vector.tensor_reduce(
                        pla[:, o0 : o0 + 256],
                        psa[:].rearrange("p (n two) -> p n two", two=2),
                        mybir.AxisListType.X,
                        mybir.AluOpType.max,
                    )
                    nc.vector.tensor_reduce(
                        plb[:, o0 : o0 + 256],
                        psb[:].rearrange("p (n two) -> p n two", two=2),
                        mybir.AxisListType.X,
                        mybir.AluOpType.max,
                    )
                    # flush pooled halves: coarse out-DMAs so they never
                    # queue ahead of the next pair's input chunks; the very
                    # last half goes out in two pieces to shorten the tail
                    last_pair = t == BPC // 2 - 1
                    if it == NT // 2 - 1:
                        spans = [(0, LOUT // 2)]
                    elif last_pair and it in (NT - 3, NT - 1):
                        spans = [(256 * (it - 1), 512)]
                    elif not last_pair and it == NT - 1:
                        spans = [(LOUT // 2, LOUT // 2)]
                    else:
                        spans = []
                    for s0, sw in spans:
                        # prelu on the pooled span (prelu commutes with max)
                        for h, pl in ((0, pla), (1, plb)):
                            b = 2 * t + h
                            o = opool.tile(
                                [128, sw], mybir.dt.bfloat16, name="o", tag="o"
                            )
                            if use_act_prelu:
                                nc.scalar.activation(
                                    o[:], pl[:, s0 : s0 + sw], AF.Prelu,
                                    alpha=slope,
                                )
                            else:
                                nc.vector.scalar_tensor_tensor(
                                    o[:], pl[:, s0 : s0 + sw], slope,
                                    pl[:, s0 : s0 + sw],
                                    mybir.AluOpType.mult, mybir.AluOpType.max,
                                )
                            nc.sync.dma_start(
                                oflat[128 * b : 128 * (b + 1), s0 : s0 + sw],
                                o[:],
                            )
    return nc


def _split_sync_waits_json(bir: bytes) -> bytes:
    """Walrus in this toolchain accepts at most one sync-wait per instruction.
    Hoist multi-wait sync_info lists into preceding single-wait EventSemaphore
    instructions on the same engine queue (the same form engine.wait_ge()
    lowers to), preserving program order and on_update placement."""
    j = json.loads(bir)
    n_split = 0
    for fn in j.get("functions", []):
        for blk in fn.get("blocks", []):
            ins_list = blk.get("instructions")
            if not ins_list:
                continue
            out = []
            for ins in ins_list:
                si = ins.get("sync_info")
                waits = si.get("on_wait") if si else None
                if waits and len(waits) > 1:
                    for i, w in enumerate(waits):
                        out.append(
                            {
                                "debug": ins.get("debug", 0),
                                "engine": ins["engine"],
                                "ins": [],
                                "outs": [],
                                "name": f"{ins['name']}-antw{i}",
                                "opcode": "EventSemaphore",
                                "sync_info": {"on_update": [], "on_wait": [w]},
                            }
                        )
                    si["on_wait"] = []
                    n_split += 1
                out.append(ins)
            blk["instructions"] = out
    return json.dumps(j).encode()


def get_program() -> "bass.Bass":
    if "nc" not in _CACHE:
        nc = build_program()
        orig = nc.to_json_bytes
        nc.to_json_bytes = lambda: _split_sync_waits_json(orig())
        _CACHE["nc"] = nc
    return _CACHE["nc"]


def prep_inputs(I, bn_gamma, bn_beta, bn_mean, bn_var, conv_w, alpha, prelu_w):
    """Host-side folding: BN -> (scale, bias); alpha -> weights; per-k lhsT
    blocks duplicated into both PE array halves."""
    f32 = np.float32
    gamma = np.asarray(bn_gamma, f32)
    beta = np.asarray(bn_beta, f32)
    mean = np.asarray(bn_mean, f32)
    var = np.asarray(bn_var, f32)
    s = gamma / np.sqrt(var + f32(BN_EPS))        # [CIN]
    t = beta - mean * s                            # [CIN]

    w = np.asarray(conv_w, f32) * np.asarray(alpha, f32)[:, None, None]  # [COUT, CIN, K]
    Wb = np.zeros((128, K * 128), np.float32)
    for k in range(K):
        Wb[0:64, 128 * k : 128 * k + 128] = w[:, :, k].T
        Wb[64:128, 128 * k : 128 * k + 128] = w[:, :, k].T
    Wb = Wb.astype(ml_dtypes.bfloat16)

    a = f32(np.asarray(prelu_w, f32).reshape(-1)[0])
    sbp = np.zeros((128, 4), f32)
    sbp[0:64, 0] = s
    sbp[64:128, 0] = s
    sbp[0:64, 1] = t
    sbp[64:128, 1] = t
    sbp[:, 2] = f32(1.0) - a
    sbp[:, 3] = a
    return Wb, sbp


def kernel(I, bn_gamma, bn_beta, bn_mean, bn_var, conv_w, alpha, prelu_w):
    I = np.ascontiguousarray(np.asarray(I, np.float32))
    assert I.shape == (B, CIN, L), I.shape
    Wb, sbp = prep_inputs(I, bn_gamma, bn_beta, bn_mean, bn_var, conv_w, alpha, prelu_w)

    nc = get_program()
    in_maps = [
        {"I8": I[BPC * c : BPC * (c + 1)], "W": Wb, "SBp": sbp} for c in range(N_CORES)
    ]
    res = run_bass_kernel_spmd(nc, in_maps, core_ids=list(range(N_CORES)))
    out = np.concatenate(
        [np.asarray(res.results[c]["O8"]) for c in range(N_CORES)], axis=0
    )
    return np.ascontiguousarray(out.astype(np.float32))



# revision 41
# speedup vs baseline: 1.0056x; 1.0056x over previous
"""Trainium2 Bass kernel: BN(eval) -> sign -> Conv1d(K=7,pad=3) -> alpha -> PReLU -> MaxPool2.

Strategy (hardcoded for B=64, CIN=64, L=4096, COUT=128, K=7):
  - Data-parallel over batch: 8 samples per NeuronCore x 8 cores; no
    cross-core communication.
  - Host folds BN into a per-channel sign threshold theta = mean - beta/s
    (sign(s*x+t) == x > theta since s>0), folds alpha AND a factor 2 into
    the bf16 conv weights; signs are materialized as +-0.5 so the DVE can
    produce them in ONE tensor_scalar instruction: (x > theta) - 0.5.
  - A PAIR of samples shares one [128, L+8] bf16 sign tile: rows 0-63 =
    sample A, rows 64-127 = sample B. Input DMA in 4 chunks of 1024+6
    cols (6-col halo) per pair; DVE signs each chunk as it lands.
  - Conv = 7 PSUM-accumulated K=64 bf16 matmuls per 512-col tile; sample
    A's matmuls run on PE row-group 0-1 and B's on 2-3 concurrently
    (weights duplicated into both halves), ~full bf16 array rate.
  - PSUM eviction fuses PReLU + MaxPool split across ACT and DVE: ScalarE
    Prelu reads PSUM fp32 and writes bf16 deinterleaved (even cols ->
    sc[0:256], odd -> sc[256:512]); VectorE tensor_tensor(max) then pools
    with both SBUF read ports at 2x (prelu commutes with max).
  - Warmup: 9 back-to-back N=512 matmuls on the weight tile run during
    the first input DMA so the PE_HAM clock gate flips to 8/8 (~2.4GHz)
    at ~4us instead of ~17us.
  - Next pair's input DMA + sign instructions are emitted mid-tile-loop
    so the DVE queue never stalls pair t+1's signs behind pair t's
    evictions; output DMAs ride the gpsimd (half A) and sync (half B)
    queues so they never delay input chunks.
  - Walrus in this toolchain accepts only one sync-wait per instruction,
    so the Tile-scheduled BIR is post-processed: multi-wait sync_info
    lists become single-wait EventSemaphore instructions (see
    _split_sync_waits_json).
"""

import json
import sys

for _p in ("/opt/trn_rl_repo", "/root/.axon_site/_ro/trn_rl_repo"):
    if _p not in sys.path:
        sys.path.append(_p)

import numpy as np
import ml_dtypes

import concourse.bass as bass
import concourse.tile as tile
from concourse import mybir
from concourse.bass_utils import run_bass_kernel_spmd

B, CIN, L, COUT, K = 64, 64, 4096, 128, 7
PAD = 3
BN_EPS = 1e-5
N_CORES = 8
BPC = B // N_CORES  # samples per core
LOUT = L // 2       # 2048 pooled length
NT = L // 512       # 8 matmul tiles of 512 cols
NCHUNK = 4
CW = L // NCHUNK    # 1024
HALO = K - 1

_CACHE: dict = {}


def build_program() -> "bass.Bass":
    nc = bass.Bass(trn_type="TRN2")
    I8 = nc.dram_tensor("I8", [BPC, CIN, L], mybir.dt.float32, kind="ExternalInput")
    W = nc.dram_tensor("W", [128, K * 128], mybir.dt.bfloat16, kind="ExternalInput")
    SBp = nc.dram_tensor("SBp", [128, 2], mybir.dt.float32, kind="ExternalInput")
    O8 = nc.dram_tensor("O8", [BPC, COUT, LOUT], mybir.dt.bfloat16, kind="ExternalOutput")

    iflat = I8.ap().flatten_outer_dims()  # [BPC*64, 4096]
    oflat = O8.ap().flatten_outer_dims()  # [BPC*128, 2048]

    AF = mybir.ActivationFunctionType
    ALU = mybir.AluOpType
    SGW = L + 8  # sign tile width: cols 0-2 zero pad, 3..L+2 data, L+3.. zero
    NPAIR = BPC // 2
    # chunk start/width (data cols): small leading chunks so the first
    # matmuls start as early as the DMA latency allows
    CH0 = [0, 512, 1024, 2048, 3072]
    CHW = [518, 518, 1030, 1030, 1024]
    NCH = len(CH0)
    with tile.TileContext(nc) as tc:
        with (
            tc.tile_pool(name="consts", bufs=1) as consts,
            tc.tile_pool(name="ipair", bufs=8) as ipool,
            tc.tile_pool(name="sgn", bufs=2) as spool,
            tc.tile_pool(name="scp", bufs=6) as scpool,
            tc.tile_pool(name="outb", bufs=4) as obpool,
            tc.tile_pool(name="ps", bufs=4, space="PSUM") as pspool,
        ):
            # the first input chunk is the longest pole at the start (its
            # DMA completion latency gates the first real matmul), so its
            # two half-DMAs go out as the FIRST instruction on the sync and
            # gpsimd rings; the weights ride the scalar HWDGE ring in
            # parallel
            # NOTE: the tile allocation order (w_sb, sb_sb, dummy, wz) fixes
            # the SBUF offsets; shifting w_sb or sg measurably slows the
            # whole matmul stream ~20% (SBUF bank phasing), so keep the
            # allocation order and only sequence the instruction emission
            w_sb = consts.tile([128, K * 128], mybir.dt.bfloat16)
            sb_sb = consts.tile([128, 2], mybir.dt.float32)
            dummy = consts.tile([128, 2], mybir.dt.float32)
            wz = consts.tile([128, 512], mybir.dt.bfloat16)
            theta = sb_sb[:, 0:1]
            slope = sb_sb[:, 1:2]

            # chunk 0 is the head-critical path: it goes out first on the
            # sync HWDGE ring (which burst-executes 128 descriptors in
            # ~0.5us once started) with W FIFO'd right behind it -- per-ring
            # FIFO means W doesn't steal SDMA round-robin slots from chunk
            # 0, and the gpsimd ring (software-DGE, ~4x slower) is avoided
            nc.gpsimd.memset(wz[:], 0.0)
            ch0 = ipool.tile([128, CW + HALO], mybir.dt.float32, name="ipc", tag="ipc")
            nc.sync.dma_start(ch0[:, 0:CHW[0]], iflat[0:128, 0 : CHW[0]])
            ch1 = ipool.tile([128, CW + HALO], mybir.dt.float32, name="ipc", tag="ipc")
            nc.sync.dma_start(ch1[:, 0 : CHW[1]], iflat[0:128, CH0[1] : CH0[1] + CHW[1]])
            nc.sync.dma_start(w_sb[:], W.ap()[:])
            nc.scalar.dma_start(sb_sb[:], SBp.ap()[:])
            # dummy activation: hoists the ACT (Prelu) table load to kernel
            # start so the first real eviction doesn't pay the table fetch
            nc.scalar.activation(dummy[:], sb_sb[:], AF.Prelu, alpha=slope)
            # PE warmup mimicking the real stream exactly (PSUM-accumulating
            # 7-tap groups on alternating row halves): isolated start/stop
            # matmuls measurably never flip the HAM clock gate, but this
            # pattern does, ~4.8us after streaming begins -- before the
            # first real matmul arrives
            for w in (7, 7, 7):
                warm = pspool.tile([128, 1024], mybir.dt.float32, name="warm", tag="psb")
                for k in range(w):
                    nc.tensor.matmul(
                        warm[:, 0:512], wz[0:64, 0:128], wz[0:64, 0:512],
                        start=(k == 0), stop=(k == w - 1),
                    )
                    nc.tensor.matmul(
                        warm[:, 512:1024], wz[64:128, 0:128], wz[64:128, 0:512],
                        start=(k == 0), stop=(k == w - 1),
                    )

            sg_t = [None] * NPAIR

            def emit_sign(t, c, ipc):
                c0, w = CH0[c], CHW[c]
                # sign/2 in one DVE op: (x > theta) - 0.5  (weights carry 2x)
                nc.vector.tensor_scalar(
                    sg_t[t][:, 3 + c0 : 3 + c0 + w],
                    ipc[:, 0:w],
                    theta, 0.5, ALU.is_gt, ALU.subtract,
                )

            def emit_in_chunk(t, c):
                """DMA input chunk c of pair t and sign it on the DVE."""
                c0, w = CH0[c], CHW[c]
                ipc = ipool.tile([128, CW + HALO], mybir.dt.float32, name="ipc", tag="ipc")
                nc.sync.dma_start(
                    ipc[:, 0:w],
                    iflat[128 * t : 128 * (t + 1), c0 : c0 + w],
                )
                emit_sign(t, c, ipc)

            def start_pair(t):
                sg = spool.tile([128, SGW], mybir.dt.bfloat16, name="sg", tag="sg")
                sg_t[t] = sg
                nc.gpsimd.memset(sg[:, 0:3], 0.0)
                nc.gpsimd.memset(sg[:, L + 3 : SGW], 0.0)

            start_pair(0)
            emit_sign(0, 0, ch0)
            emit_sign(0, 1, ch1)
            for c in range(2, NCH):
                emit_in_chunk(0, c)

            for t in range(NPAIR):
                sg = sg_t[t]
                oba = obpool.tile([128, LOUT], mybir.dt.bfloat16, name="oba", tag="oba")
                obb = obpool.tile([128, LOUT], mybir.dt.bfloat16, name="obb", tag="obb")
                for it in range(NT):
                    # the final tile uses two separate PSUM tiles so its
                    # ACT and DVE evictions aren't reader-serialized (Tile
                    # chains readers of one tile)
                    if t == NPAIR - 1 and it == NT - 1:
                        psa = pspool.tile([128, 512], mybir.dt.float32, name="psl", tag="psb")[:]
                        psb = pspool.tile([128, 512], mybir.dt.float32, name="psl2", tag="psb")[:]
                    else:
                        ps2 = pspool.tile([128, 1024], mybir.dt.float32, name="ps2", tag="psb")
                        psa = ps2[:, 0:512]
                        psb = ps2[:, 512:1024]
                    for k in range(K):
                        c0 = 512 * it + k
                        nc.tensor.matmul(
                            psa, w_sb[0:64, 128 * k : 128 * (k + 1)],
                            sg[0:64, c0 : c0 + 512],
                            start=(k == 0), stop=(k == K - 1),
                        )
                        nc.tensor.matmul(
                            psb, w_sb[64:128, 128 * k : 128 * (k + 1)],
                            sg[64:128, c0 : c0 + 512],
                            start=(k == 0), stop=(k == K - 1),
                        )
                    o0 = 256 * it
                    # ScalarE PReLU evicts both samples' PSUM banks in one
                    # 1024-wide activation (plain sequential write; strided
                    # ACT writes measure ~5x slower), then DVE pools each
                    # half via even/odd strided tensor_tensor (both read
                    # ports in parallel -> ~output-elems cycles)
                    sc = scpool.tile([128, 1024], mybir.dt.bfloat16, name="sc", tag="sc")
                    last_tile = t == NPAIR - 1 and it == NT - 1
                    if last_tile:
                        # parallel eviction of the final tile: half B pools
                        # on the DVE straight from PSUM while ACT prelus
                        # half A, so both flush chains finish ~together.
                        # Separate tiles: Tile serializes same-tile writers.
                        sc2 = consts.tile([128, 256], mybir.dt.bfloat16)
                        nc.vector.tensor_reduce(
                            sc2[:],
                            psb.rearrange("p (n two) -> p n two", two=2),
                            mybir.AxisListType.X,
                            ALU.max,
                        )
                        nc.scalar.activation(
                            sc[:, 0:512], psa, AF.Prelu, alpha=slope
                        )
                        scv = sc[:, 0:512].rearrange("p (j k) -> p j k", k=2)
                        nc.vector.tensor_tensor(
                            oba[:, o0 : o0 + 256],
                            scv[:, :, 0], scv[:, :, 1],
                            ALU.max,
                        )
                        nc.scalar.activation(
                            obb[:, o0 : o0 + 256], sc2[:],
                            AF.Prelu, alpha=slope,
                        )
                    else:
                        nc.scalar.activation(sc[:], ps2[:], AF.Prelu, alpha=slope)
                        for h, ob in ((0, oba), (1, obb)):
                            scv = sc[:, 512 * h : 512 * h + 512].rearrange(
                                "p (j k) -> p j k", k=2
                            )
                            nc.vector.tensor_tensor(
                                ob[:, o0 : o0 + 256],
                                scv[:, :, 0], scv[:, :, 1],
                                ALU.max,
                            )
                    # software-pipeline the NEXT pair's input + signs so the
                    # DVE queue has them before this pair's evictions finish
                    if t + 1 < NPAIR:
                        if it == 1:
                            start_pair(t + 1)
                        if 2 <= it < 2 + NCH:
                            emit_in_chunk(t + 1, it - 2)
                    # flush outputs every 2 tiles (per tile at the very end
                    # to shorten the DMA tail); half A rides the gpsimd DMA
                    # queue, half B the sync queue, so outputs never delay
                    # the next pair's input chunks
                    fin = t == NPAIR - 1 and it >= NT - 2
                    if it % 2 == 1 or fin:
                        s0 = 256 * it if fin else 256 * (it - 1)
                        sw = 256 if fin else 512
                        # the final flushes both ride the fast sync ring
                        a_dma = nc.sync.dma_start if fin else nc.gpsimd.dma_start
                        a_dma(
                            oflat[128 * (2 * t) : 128 * (2 * t) + 128, s0 : s0 + sw],
                            oba[:, s0 : s0 + sw],
                        )
                        nc.sync.dma_start(
                            oflat[128 * (2 * t + 1) : 128 * (2 * t + 1) + 128, s0 : s0 + sw],
                            obb[:, s0 : s0 + sw],
                        )
    return nc


def _split_sync_waits_json(bir: bytes) -> bytes:
    """Walrus in this toolchain accepts at most one sync-wait per instruction.
    Hoist multi-wait sync_info lists into preceding single-wait EventSemaphore
    instructions on the same engine queue (the same form engine.wait_ge()
    lowers to), preserving program order and on_update placement."""
    j = json.loads(bir)
    n_split = 0
    for fn in j.get("functions", []):
        for blk in fn.get("blocks", []):
            ins_list = blk.get("instructions")
            if not ins_list:
                continue
            out = []
            for ins in ins_list:
                si = ins.get("sync_info")
                waits = si.get("on_wait") if si else None
                if waits and len(waits) > 1:
                    for i, w in enumerate(waits):
                        out.append(
                            {
                                "debug": ins.get("debug", 0),
                                "engine": ins["engine"],
                                "ins": [],
                                "outs": [],
                                "name": f"{ins['name']}-antw{i}",
                                "opcode": "EventSemaphore",
                                "sync_info": {"on_update": [], "on_wait": [w]},
                            }
                        )
                    si["on_wait"] = []
                    n_split += 1
                out.append(ins)
            blk["instructions"] = out
    return json.dumps(j).encode()


def get_program() -> "bass.Bass":
    if "nc" not in _CACHE:
        nc = build_program()
        orig = nc.to_json_bytes
        nc.to_json_bytes = lambda: _split_sync_waits_json(orig())
        _CACHE["nc"] = nc
    return _CACHE["nc"]


def prep_inputs(I, bn_gamma, bn_beta, bn_mean, bn_var, conv_w, alpha, prelu_w):
    """Host-side folding: BN -> per-channel sign threshold; alpha and the
    +-0.5 sign representation (factor 2) -> weights; per-k lhsT blocks
    duplicated into both PE array halves."""
    f32 = np.float32
    gamma = np.asarray(bn_gamma, f32)
    beta = np.asarray(bn_beta, f32)
    mean = np.asarray(bn_mean, f32)
    var = np.asarray(bn_var, f32)
    s = gamma / np.sqrt(var + f32(BN_EPS))        # [CIN]
    theta = mean - beta / s                        # sign(s*x+t) == x > theta

    w = 2.0 * np.asarray(conv_w, f32) * np.asarray(alpha, f32)[:, None, None]
    Wb = np.zeros((128, K * 128), np.float32)
    for k in range(K):
        Wb[0:64, 128 * k : 128 * k + 128] = w[:, :, k].T
        Wb[64:128, 128 * k : 128 * k + 128] = w[:, :, k].T
    Wb = Wb.astype(ml_dtypes.bfloat16)

    a = f32(np.asarray(prelu_w, f32).reshape(-1)[0])
    sbp = np.zeros((128, 2), f32)
    sbp[0:64, 0] = theta
    sbp[64:128, 0] = theta
    sbp[:, 1] = a
    return Wb, sbp


def kernel(I, bn_gamma, bn_beta, bn_mean, bn_var, conv_w, alpha, prelu_w):
    I = np.ascontiguousarray(np.asarray(I, np.float32))
    assert I.shape == (B, CIN, L), I.shape
    Wb, sbp = prep_inputs(I, bn_gamma, bn_beta, bn_mean, bn_var, conv_w, alpha, prelu_w)

    nc = get_program()
    in_maps = [
        {"I8": I[BPC * c : BPC * (c + 1)], "W": Wb, "SBp": sbp} for c in range(N_CORES)
    ]
    res = run_bass_kernel_spmd(nc, in_maps, core_ids=list(range(N_CORES)))
    out = np.concatenate(
        [np.asarray(res.results[c]["O8"]) for c in range(N_CORES)], axis=0
    )
    return np.ascontiguousarray(out.astype(np.float32))


# revision 43
# speedup vs baseline: 1.0113x; 1.0057x over previous
"""Trainium2 Bass kernel: BN(eval) -> sign -> Conv1d(K=7,pad=3) -> alpha -> PReLU -> MaxPool2.

Strategy (hardcoded for B=64, CIN=64, L=4096, COUT=128, K=7):
  - Data-parallel over batch: 8 samples per NeuronCore x 8 cores; no
    cross-core communication.
  - Host folds BN into a per-channel sign threshold theta = mean - beta/s
    (sign(s*x+t) == x > theta since s>0), folds alpha AND a factor 2 into
    the bf16 conv weights; signs are materialized as +-0.5 so the DVE can
    produce them in ONE tensor_scalar instruction: (x > theta) - 0.5.
  - A PAIR of samples shares one [128, L+8] bf16 sign tile: rows 0-63 =
    sample A, rows 64-127 = sample B. Input DMA in 5 chunks (two 518-col
    leaders so the first tiles unblock at the DMA pipe's ~5.5us spin-up
    latency, then 1030-col chunks); DVE signs each chunk as it lands.
  - Conv = 7 PSUM-accumulated K=64 bf16 matmuls per 512-col tile; sample
    A's matmuls run on PE row-group 0-1 and B's on 2-3 concurrently
    (weights duplicated into both halves). The measured warm cadence is
    ~215ns per A/B matmul pair = the 78.6 TF/s bf16 roofline; the whole
    448-matmul stream runs in ~48.6us with no internal gaps.
  - psa/psb live in ONE [128,1024] 2-bank PSUM tile so a single ScalarE
    Prelu activation evicts both samples (1024 fp32 reads, ~1.2us);
    VectorE then pools each half via even/odd strided tensor_tensor(max)
    (~420ns: tensor_tensor drives both DVE read ports in parallel, and
    prelu commutes with max). ACT ~9.4us/pair and DVE ~9.7us/pair both
    hide under the 11.9us/pair tensor pace.
  - Warmup: 42 matmuls that mimic the real stream (PSUM-accumulating
    7-tap groups on alternating row halves) run off a memset tile during
    the input DMA spin-up; this pattern reliably flips the PE_HAM clock
    gate to 8/8 (2.4GHz) before the first real matmul, and the warmup
    length dovetails into the real stream so no >1us PE idle gap lets
    HAM re-throttle. Isolated start/stop warmup matmuls do NOT flip HAM.
  - Next pair's input DMA + sign instructions are emitted mid-tile-loop
    so the DVE queue never stalls pair t+1's signs behind pair t's
    evictions; output DMAs ride the gpsimd (half A) and sync (half B)
    queues so they never delay input chunks. The final tile evicts via
    parallel ACT/DVE chains on separate PSUM tiles (Tile chains readers
    of one tile) and flushes on the fast sync HWDGE ring.
  - SBUF layout note: the consts allocation order (w_sb, sb_sb, dummy,
    wz) is load-bearing -- shifting w_sb/sg offsets measurably slows the
    matmul stream ~20% (SBUF bank phasing).
  - Walrus in this toolchain accepts only one sync-wait per instruction,
    so the Tile-scheduled BIR is post-processed: multi-wait sync_info
    lists become single-wait EventSemaphore instructions (see
    _split_sync_waits_json).
"""

import json
import sys

for _p in ("/opt/trn_rl_repo", "/root/.axon_site/_ro/trn_rl_repo"):
    if _p not in sys.path:
        sys.path.append(_p)

import numpy as np
import ml_dtypes

import concourse.bass as bass
import concourse.tile as tile
from concourse import mybir
from concourse.bass_utils import run_bass_kernel_spmd

B, CIN, L, COUT, K = 64, 64, 4096, 128, 7
PAD = 3
BN_EPS = 1e-5
N_CORES = 8
BPC = B // N_CORES  # samples per core
LOUT = L // 2       # 2048 pooled length
NT = L // 512       # 8 matmul tiles of 512 cols
NCHUNK = 4
CW = L // NCHUNK    # 1024
HALO = K - 1

_CACHE: dict = {}


def build_program() -> "bass.Bass":
    nc = bass.Bass(trn_type="TRN2")
    I8 = nc.dram_tensor("I8", [BPC, CIN, L], mybir.dt.float32, kind="ExternalInput")
    W = nc.dram_tensor("W", [128, K * 128], mybir.dt.bfloat16, kind="ExternalInput")
    SBp = nc.dram_tensor("SBp", [128, 2], mybir.dt.float32, kind="ExternalInput")
    O8 = nc.dram_tensor("O8", [BPC, COUT, LOUT], mybir.dt.bfloat16, kind="ExternalOutput")

    iflat = I8.ap().flatten_outer_dims()  # [BPC*64, 4096]
    oflat = O8.ap().flatten_outer_dims()  # [BPC*128, 2048]

    AF = mybir.ActivationFunctionType
    ALU = mybir.AluOpType
    SGW = L + 8  # sign tile width: cols 0-2 zero pad, 3..L+2 data, L+3.. zero
    NPAIR = BPC // 2
    # chunk start/width (data cols): small leading chunks so the first
    # matmuls start as early as the DMA latency allows
    CH0 = [0, 512, 1024, 2048, 3072]
    CHW = [518, 518, 1030, 1030, 1024]
    NCH = len(CH0)
    with tile.TileContext(nc) as tc:
        with (
            tc.tile_pool(name="consts", bufs=1) as consts,
            tc.tile_pool(name="ipair", bufs=8) as ipool,
            tc.tile_pool(name="sgn", bufs=2) as spool,
            tc.tile_pool(name="scp", bufs=6) as scpool,
            tc.tile_pool(name="outb", bufs=4) as obpool,
            tc.tile_pool(name="ps", bufs=4, space="PSUM") as pspool,
        ):
            # the first input chunk is the longest pole at the start (its
            # DMA completion latency gates the first real matmul), so its
            # two half-DMAs go out as the FIRST instruction on the sync and
            # gpsimd rings; the weights ride the scalar HWDGE ring in
            # parallel
            # NOTE: the tile allocation order (w_sb, sb_sb, dummy, wz) fixes
            # the SBUF offsets; shifting w_sb or sg measurably slows the
            # whole matmul stream ~20% (SBUF bank phasing), so keep the
            # allocation order and only sequence the instruction emission
            w_sb = consts.tile([128, K * 128], mybir.dt.bfloat16)
            sb_sb = consts.tile([128, 2], mybir.dt.float32)
            dummy = consts.tile([128, 2], mybir.dt.float32)
            wz = consts.tile([128, 512], mybir.dt.bfloat16)
            theta = sb_sb[:, 0:1]
            slope = sb_sb[:, 1:2]

            # chunk 0 is the head-critical path: it goes out first on the
            # sync HWDGE ring (which burst-executes 128 descriptors in
            # ~0.5us once started) with W FIFO'd right behind it -- per-ring
            # FIFO means W doesn't steal SDMA round-robin slots from chunk
            # 0, and the gpsimd ring (software-DGE, ~4x slower) is avoided
            nc.gpsimd.memset(wz[:], 0.0)
            ch0 = ipool.tile([128, CW + HALO], mybir.dt.float32, name="ipc", tag="ipc")
            nc.sync.dma_start(ch0[:, 0:CHW[0]], iflat[0:128, 0 : CHW[0]])
            ch1 = ipool.tile([128, CW + HALO], mybir.dt.float32, name="ipc", tag="ipc")
            nc.sync.dma_start(ch1[:, 0 : CHW[1]], iflat[0:128, CH0[1] : CH0[1] + CHW[1]])
            nc.sync.dma_start(w_sb[:], W.ap()[:])
            nc.scalar.dma_start(sb_sb[:], SBp.ap()[:])
            # dummy activation: hoists the ACT (Prelu) table load to kernel
            # start so the first real eviction doesn't pay the table fetch
            nc.scalar.activation(dummy[:], sb_sb[:], AF.Prelu, alpha=slope)
            # PE warmup mimicking the real stream exactly (PSUM-accumulating
            # 7-tap groups on alternating row halves): isolated start/stop
            # matmuls measurably never flip the HAM clock gate, but this
            # pattern does, ~4.8us after streaming begins -- before the
            # first real matmul arrives
            for w in (7, 7, 7):
                warm = pspool.tile([128, 1024], mybir.dt.float32, name="warm", tag="psb")
                for k in range(w):
                    nc.tensor.matmul(
                        warm[:, 0:512], wz[0:64, 0:128], wz[0:64, 0:512],
                        start=(k == 0), stop=(k == w - 1),
                    )
                    nc.tensor.matmul(
                        warm[:, 512:1024], wz[64:128, 0:128], wz[64:128, 0:512],
                        start=(k == 0), stop=(k == w - 1),
                    )

            sg_t = [None] * NPAIR

            def emit_sign(t, c, ipc):
                c0, w = CH0[c], CHW[c]
                # sign/2 in one DVE op: (x > theta) - 0.5  (weights carry 2x)
                nc.vector.tensor_scalar(
                    sg_t[t][:, 3 + c0 : 3 + c0 + w],
                    ipc[:, 0:w],
                    theta, 0.5, ALU.is_gt, ALU.subtract,
                )

            def emit_in_chunk(t, c):
                """DMA input chunk c of pair t and sign it on the DVE."""
                c0, w = CH0[c], CHW[c]
                ipc = ipool.tile([128, CW + HALO], mybir.dt.float32, name="ipc", tag="ipc")
                nc.sync.dma_start(
                    ipc[:, 0:w],
                    iflat[128 * t : 128 * (t + 1), c0 : c0 + w],
                )
                emit_sign(t, c, ipc)

            def start_pair(t):
                sg = spool.tile([128, SGW], mybir.dt.bfloat16, name="sg", tag="sg")
                sg_t[t] = sg
                nc.gpsimd.memset(sg[:, 0:3], 0.0)
                nc.gpsimd.memset(sg[:, L + 3 : SGW], 0.0)

            start_pair(0)
            emit_sign(0, 0, ch0)
            emit_sign(0, 1, ch1)
            for c in range(2, NCH):
                emit_in_chunk(0, c)

            for t in range(NPAIR):
                sg = sg_t[t]
                oba = obpool.tile([128, LOUT], mybir.dt.bfloat16, name="oba", tag="oba")
                obb = obpool.tile([128, LOUT], mybir.dt.bfloat16, name="obb", tag="obb")
                for it in range(NT):
                    # the final tile uses two separate PSUM tiles so its
                    # ACT and DVE evictions aren't reader-serialized (Tile
                    # chains readers of one tile)
                    if t == NPAIR - 1 and it == NT - 1:
                        psa = pspool.tile([128, 512], mybir.dt.float32, name="psl", tag="psb")[:]
                        psb = pspool.tile([128, 512], mybir.dt.float32, name="psl2", tag="psb")[:]
                    else:
                        ps2 = pspool.tile([128, 1024], mybir.dt.float32, name="ps2", tag="psb")
                        psa = ps2[:, 0:512]
                        psb = ps2[:, 512:1024]
                    for k in range(K):
                        c0 = 512 * it + k
                        nc.tensor.matmul(
                            psa, w_sb[0:64, 128 * k : 128 * (k + 1)],
                            sg[0:64, c0 : c0 + 512],
                            start=(k == 0), stop=(k == K - 1),
                        )
                        nc.tensor.matmul(
                            psb, w_sb[64:128, 128 * k : 128 * (k + 1)],
                            sg[64:128, c0 : c0 + 512],
                            start=(k == 0), stop=(k == K - 1),
                        )
                    o0 = 256 * it
                    # ScalarE PReLU evicts both samples' PSUM banks in one
                    # 1024-wide activation (plain sequential write; strided
                    # ACT writes measure ~5x slower), then DVE pools each
                    # half via even/odd strided tensor_tensor (both read
                    # ports in parallel -> ~output-elems cycles)
                    sc = scpool.tile([128, 1024], mybir.dt.bfloat16, name="sc", tag="sc")
                    last_tile = t == NPAIR - 1 and it == NT - 1
                    if last_tile:
                        # parallel eviction of the final tile: half B pools
                        # on the DVE straight from PSUM while ACT prelus
                        # half A, so both flush chains finish ~together.
                        # Separate tiles: Tile serializes same-tile writers.
                        sc2 = consts.tile([128, 256], mybir.dt.bfloat16)
                        nc.vector.tensor_reduce(
                            sc2[:],
                            psb.rearrange("p (n two) -> p n two", two=2),
                            mybir.AxisListType.X,
                            ALU.max,
                        )
                        nc.scalar.activation(
                            sc[:, 0:512], psa, AF.Prelu, alpha=slope
                        )
                        scv = sc[:, 0:512].rearrange("p (j k) -> p j k", k=2)
                        nc.vector.tensor_tensor(
                            oba[:, o0 : o0 + 256],
                            scv[:, :, 0], scv[:, :, 1],
                            ALU.max,
                        )
                        nc.scalar.activation(
                            obb[:, o0 : o0 + 256], sc2[:],
                            AF.Prelu, alpha=slope,
                        )
                    else:
                        nc.scalar.activation(sc[:], ps2[:], AF.Prelu, alpha=slope)
                        for h, ob in ((0, oba), (1, obb)):
                            scv = sc[:, 512 * h : 512 * h + 512].rearrange(
                                "p (j k) -> p j k", k=2
                            )
                            nc.vector.tensor_tensor(
                                ob[:, o0 : o0 + 256],
                                scv[:, :, 0], scv[:, :, 1],
                                ALU.max,
                            )
                    # software-pipeline the NEXT pair's input + signs so the
                    # DVE queue has them before this pair's evictions finish
                    if t + 1 < NPAIR:
                        if it == 1:
                            start_pair(t + 1)
                        if 2 <= it < 2 + NCH:
                            emit_in_chunk(t + 1, it - 2)
                    # flush outputs every 2 tiles (per tile at the very end
                    # to shorten the DMA tail); half A rides the gpsimd DMA
                    # queue, half B the sync queue, so outputs never delay
                    # the next pair's input chunks
                    fin = t == NPAIR - 1 and it >= NT - 2
                    if it % 2 == 1 or fin:
                        s0 = 256 * it if fin else 256 * (it - 1)
                        sw = 256 if fin else 512
                        # only the very last tile's flushes ride the fast
                        # sync ring (tile 6's half A has slack; keeping it
                        # on gpsimd avoids serializing 4 issue slots on
                        # sync right at the end)
                        a_dma = (
                            nc.sync.dma_start
                            if fin and it == NT - 1
                            else nc.gpsimd.dma_start
                        )
                        a_dma(
                            oflat[128 * (2 * t) : 128 * (2 * t) + 128, s0 : s0 + sw],
                            oba[:, s0 : s0 + sw],
                        )
                        nc.sync.dma_start(
                            oflat[128 * (2 * t + 1) : 128 * (2 * t + 1) + 128, s0 : s0 + sw],
                            obb[:, s0 : s0 + sw],
                        )
    return nc


def _split_sync_waits_json(bir: bytes) -> bytes:
    """Walrus in this toolchain accepts at most one sync-wait per instruction.
    Hoist multi-wait sync_info lists into preceding single-wait EventSemaphore
    instructions on the same engine queue (the same form engine.wait_ge()
    lowers to), preserving program order and on_update placement."""
    j = json.loads(bir)
    n_split = 0
    for fn in j.get("functions", []):
        for blk in fn.get("blocks", []):
            ins_list = blk.get("instructions")
            if not ins_list:
                continue
            out = []
            for ins in ins_list:
                si = ins.get("sync_info")
                waits = si.get("on_wait") if si else None
                if waits and len(waits) > 1:
                    for i, w in enumerate(waits):
                        out.append(
                            {
                                "debug": ins.get("debug", 0),
                                "engine": ins["engine"],
                                "ins": [],
                                "outs": [],
                                "name": f"{ins['name']}-antw{i}",
                                "opcode": "EventSemaphore",
                                "sync_info": {"on_update": [], "on_wait": [w]},
                            }
                        )
                    si["on_wait"] = []
                    n_split += 1
                out.append(ins)
            blk["instructions"] = out
    return json.dumps(j).encode()


def get_program() -> "bass.Bass":
    if "nc" not in _CACHE:
        nc = build_program()
        orig = nc.to_json_bytes
        nc.to_json_bytes = lambda: _split_sync_waits_json(orig())
        _CACHE["nc"] = nc
    return _CACHE["nc"]


def prep_inputs(I, bn_gamma, bn_beta, bn_mean, bn_var, conv_w, alpha, prelu_w):
    """Host-side folding: BN -> per-channel sign threshold; alpha and the
    +-0.5 sign representation (factor 2) -> weights; per-k lhsT blocks
    duplicated into both PE array halves."""
    f32 = np.float32
    gamma = np.asarray(bn_gamma, f32)
    beta = np.asarray(bn_beta, f32)
    mean = np.asarray(bn_mean, f32)
    var = np.asarray(bn_var, f32)
    s = gamma / np.sqrt(var + f32(BN_EPS))        # [CIN]
    theta = mean - beta / s                        # sign(s*x+t) == x > theta

    w = 2.0 * np.asarray(conv_w, f32) * np.asarray(alpha, f32)[:, None, None]
    Wb = np.zeros((128, K * 128), np.float32)
    for k in range(K):
        Wb[0:64, 128 * k : 128 * k + 128] = w[:, :, k].T
        Wb[64:128, 128 * k : 128 * k + 128] = w[:, :, k].T
    Wb = Wb.astype(ml_dtypes.bfloat16)

    a = f32(np.asarray(prelu_w, f32).reshape(-1)[0])
    sbp = np.zeros((128, 2), f32)
    sbp[0:64, 0] = theta
    sbp[64:128, 0] = theta
    sbp[:, 1] = a
    return Wb, sbp


def kernel(I, bn_gamma, bn_beta, bn_mean, bn_var, conv_w, alpha, prelu_w):
    I = np.ascontiguousarray(np.asarray(I, np.float32))
    assert I.shape == (B, CIN, L), I.shape
    Wb, sbp = prep_inputs(I, bn_gamma, bn_beta, bn_mean, bn_var, conv_w, alpha, prelu_w)

    nc = get_program()
    in_maps = [
        {"I8": I[BPC * c : BPC * (c + 1)], "W": Wb, "SBp": sbp} for c in range(N_CORES)
    ]
    res = run_bass_kernel_spmd(nc, in_maps, core_ids=list(range(N_CORES)))
    out = np.concatenate(
        [np.asarray(res.results[c]["O8"]) for c in range(N_CORES)], axis=0
    )
    return np.ascontiguousarray(out.astype(np.float32))


# revision 45
# speedup vs baseline: 1.0304x; 1.0189x over previous
"""Trainium2 Bass kernel: BN(eval) -> sign -> Conv1d(K=7,pad=3) -> alpha -> PReLU -> MaxPool2.

Strategy (hardcoded for B=64, CIN=64, L=4096, COUT=128, K=7):
  - Data-parallel over batch: 8 samples per NeuronCore x 8 cores; no
    cross-core communication.
  - Host folds BN into a per-channel sign threshold theta = mean - beta/s
    (sign(s*x+t) == x > theta since s>0), folds alpha AND a factor 2 into
    the bf16 conv weights; signs are materialized as +-0.5 so the DVE can
    produce them in ONE tensor_scalar instruction: (x > theta) - 0.5.
  - A PAIR of samples shares one [128, L+8] bf16 sign tile: rows 0-63 =
    sample A, rows 64-127 = sample B. Input DMA in 5 chunks (two 518-col
    leaders so the first tiles unblock at the DMA pipe's ~5.5us spin-up
    latency, then 1030-col chunks); DVE signs each chunk as it lands.
  - Conv = 7 PSUM-accumulated K=64 bf16 matmuls per 512-col tile; sample
    A's matmuls run on PE row-group 0-1 and B's on 2-3 concurrently
    (weights duplicated into both halves). The measured warm cadence is
    ~215ns per A/B matmul pair = the 78.6 TF/s bf16 roofline; the whole
    448-matmul stream runs in ~48.6us with no internal gaps.
  - psa/psb live in ONE [128,1024] 2-bank PSUM tile so a single ScalarE
    Prelu activation evicts both samples (1024 fp32 reads, ~1.2us);
    VectorE then pools each half via even/odd strided tensor_tensor(max)
    (~420ns: tensor_tensor drives both DVE read ports in parallel, and
    prelu commutes with max). ACT ~9.4us/pair and DVE ~9.7us/pair both
    hide under the 11.9us/pair tensor pace.
  - Warmup: 42 matmuls that mimic the real stream (PSUM-accumulating
    7-tap groups on alternating row halves) run off a memset tile during
    the input DMA spin-up; this pattern reliably flips the PE_HAM clock
    gate to 8/8 (2.4GHz) before the first real matmul, and the warmup
    length dovetails into the real stream so no >1us PE idle gap lets
    HAM re-throttle. Isolated start/stop warmup matmuls do NOT flip HAM.
  - Next pair's input DMA + sign instructions are emitted mid-tile-loop
    so the DVE queue never stalls pair t+1's signs behind pair t's
    evictions; output DMAs ride the gpsimd (half A) and sync (half B)
    queues so they never delay input chunks. The final tile evicts via
    parallel ACT/DVE chains on separate PSUM tiles (Tile chains readers
    of one tile) and flushes on the fast sync HWDGE ring.
  - SBUF layout note: the consts allocation order (w_sb, sb_sb, dummy,
    wz) is load-bearing -- shifting w_sb/sg offsets measurably slows the
    matmul stream ~20% (SBUF bank phasing).
  - Walrus in this toolchain accepts only one sync-wait per instruction,
    so the Tile-scheduled BIR is post-processed: multi-wait sync_info
    lists become single-wait EventSemaphore instructions (see
    _split_sync_waits_json).
"""

import json
import sys

for _p in ("/opt/trn_rl_repo", "/root/.axon_site/_ro/trn_rl_repo"):
    if _p not in sys.path:
        sys.path.append(_p)

import numpy as np
import ml_dtypes

import concourse.bass as bass
import concourse.tile as tile
from concourse import mybir
from concourse.bass_utils import run_bass_kernel_spmd

B, CIN, L, COUT, K = 64, 64, 4096, 128, 7
PAD = 3
BN_EPS = 1e-5
N_CORES = 8
BPC = B // N_CORES  # samples per core
LOUT = L // 2       # 2048 pooled length
NT = L // 512       # 8 matmul tiles of 512 cols
NCHUNK = 4
CW = L // NCHUNK    # 1024
HALO = K - 1

_CACHE: dict = {}


def build_program() -> "bass.Bass":
    nc = bass.Bass(trn_type="TRN2")
    I8 = nc.dram_tensor("I8", [BPC, CIN, L], mybir.dt.float32, kind="ExternalInput")
    W = nc.dram_tensor("W", [128, K * 128], mybir.dt.bfloat16, kind="ExternalInput")
    SBp = nc.dram_tensor("SBp", [128, 2], mybir.dt.float32, kind="ExternalInput")
    O8 = nc.dram_tensor("O8", [BPC, COUT, LOUT], mybir.dt.bfloat16, kind="ExternalOutput")

    iflat = I8.ap().flatten_outer_dims()  # [BPC*64, 4096]
    oflat = O8.ap().flatten_outer_dims()  # [BPC*128, 2048]

    AF = mybir.ActivationFunctionType
    ALU = mybir.AluOpType
    SGW = L + 8  # sign tile width: cols 0-2 zero pad, 3..L+2 data, L+3.. zero
    NPAIR = BPC // 2
    # chunk start/width (data cols): small leading chunks so the first
    # matmuls start as early as the DMA latency allows
    CH0 = [0, 512, 1024, 2048, 3072]
    CHW = [518, 518, 1030, 1030, 1024]
    NCH = len(CH0)
    with tile.TileContext(nc) as tc:
        with (
            tc.tile_pool(name="consts", bufs=1) as consts,
            tc.tile_pool(name="ipair", bufs=8) as ipool,
            tc.tile_pool(name="sgn", bufs=2) as spool,
            tc.tile_pool(name="scp", bufs=6) as scpool,
            tc.tile_pool(name="outb", bufs=4) as obpool,
            tc.tile_pool(name="ps", bufs=4, space="PSUM") as pspool,
        ):
            # the first input chunk is the longest pole at the start (its
            # DMA completion latency gates the first real matmul), so its
            # two half-DMAs go out as the FIRST instruction on the sync and
            # gpsimd rings; the weights ride the scalar HWDGE ring in
            # parallel
            # NOTE: the tile allocation order (w_sb, sb_sb, dummy, wz) fixes
            # the SBUF offsets; shifting w_sb or sg measurably slows the
            # whole matmul stream ~20% (SBUF bank phasing), so keep the
            # allocation order and only sequence the instruction emission
            w_sb = consts.tile([128, K * 128], mybir.dt.bfloat16)
            sb_sb = consts.tile([128, 2], mybir.dt.float32)
            dummy = consts.tile([128, 2], mybir.dt.float32)
            wz = consts.tile([128, 512], mybir.dt.bfloat16)
            theta = sb_sb[:, 0:1]
            slope = sb_sb[:, 1:2]

            # chunk 0 is the head-critical path: it goes out first on the
            # sync HWDGE ring (which burst-executes 128 descriptors in
            # ~0.5us once started) with W FIFO'd right behind it -- per-ring
            # FIFO means W doesn't steal SDMA round-robin slots from chunk
            # 0, and the gpsimd ring (software-DGE, ~4x slower) is avoided
            nc.gpsimd.memset(wz[:], 0.0)
            ch0 = ipool.tile([128, CW + HALO], mybir.dt.float32, name="ipc", tag="ipc")
            nc.sync.dma_start(ch0[:, 0:CHW[0]], iflat[0:128, 0 : CHW[0]])
            ch1 = ipool.tile([128, CW + HALO], mybir.dt.float32, name="ipc", tag="ipc")
            nc.sync.dma_start(ch1[:, 0 : CHW[1]], iflat[0:128, CH0[1] : CH0[1] + CHW[1]])
            nc.scalar.dma_start(w_sb[:], W.ap()[:])
            nc.scalar.dma_start(sb_sb[:], SBp.ap()[:])
            # dummy activation: hoists the ACT (Prelu) table load to kernel
            # start so the first real eviction doesn't pay the table fetch
            nc.scalar.activation(dummy[:], sb_sb[:], AF.Prelu, alpha=slope)
            # PE warmup mimicking the real stream exactly (PSUM-accumulating
            # 7-tap groups on alternating row halves): isolated start/stop
            # matmuls measurably never flip the HAM clock gate, but this
            # pattern does, ~4.8us after streaming begins -- before the
            # first real matmul arrives
            for w in (7, 7, 3):
                warm = pspool.tile([128, 1024], mybir.dt.float32, name="warm", tag="psb")
                for k in range(w):
                    nc.tensor.matmul(
                        warm[:, 0:512], wz[0:64, 0:128], wz[0:64, 0:512],
                        start=(k == 0), stop=(k == w - 1),
                    )
                    nc.tensor.matmul(
                        warm[:, 512:1024], wz[64:128, 0:128], wz[64:128, 0:512],
                        start=(k == 0), stop=(k == w - 1),
                    )

            sg_t = [None] * NPAIR

            def emit_sign(t, c, ipc):
                c0, w = CH0[c], CHW[c]
                # sign/2 in one DVE op: (x > theta) - 0.5  (weights carry 2x)
                nc.vector.tensor_scalar(
                    sg_t[t][:, 3 + c0 : 3 + c0 + w],
                    ipc[:, 0:w],
                    theta, 0.5, ALU.is_gt, ALU.subtract,
                )

            def emit_in_chunk(t, c):
                """DMA input chunk c of pair t and sign it on the DVE."""
                c0, w = CH0[c], CHW[c]
                ipc = ipool.tile([128, CW + HALO], mybir.dt.float32, name="ipc", tag="ipc")
                nc.sync.dma_start(
                    ipc[:, 0:w],
                    iflat[128 * t : 128 * (t + 1), c0 : c0 + w],
                )
                emit_sign(t, c, ipc)

            def start_pair(t):
                sg = spool.tile([128, SGW], mybir.dt.bfloat16, name="sg", tag="sg")
                sg_t[t] = sg
                nc.gpsimd.memset(sg[:, 0:3], 0.0)
                nc.gpsimd.memset(sg[:, L + 3 : SGW], 0.0)

            start_pair(0)
            emit_sign(0, 0, ch0)
            emit_sign(0, 1, ch1)
            for c in range(2, NCH):
                emit_in_chunk(0, c)

            for t in range(NPAIR):
                sg = sg_t[t]
                oba = obpool.tile([128, LOUT], mybir.dt.bfloat16, name="oba", tag="oba")
                obb = obpool.tile([128, LOUT], mybir.dt.bfloat16, name="obb", tag="obb")
                for it in range(NT):
                    # the final tile uses two separate PSUM tiles so its
                    # ACT and DVE evictions aren't reader-serialized (Tile
                    # chains readers of one tile)
                    if t == NPAIR - 1 and it == NT - 1:
                        psa = pspool.tile([128, 512], mybir.dt.float32, name="psl", tag="psb")[:]
                        psb = pspool.tile([128, 512], mybir.dt.float32, name="psl2", tag="psb")[:]
                    else:
                        ps2 = pspool.tile([128, 1024], mybir.dt.float32, name="ps2", tag="psb")
                        psa = ps2[:, 0:512]
                        psb = ps2[:, 512:1024]
                    for k in range(K):
                        c0 = 512 * it + k
                        nc.tensor.matmul(
                            psa, w_sb[0:64, 128 * k : 128 * (k + 1)],
                            sg[0:64, c0 : c0 + 512],
                            start=(k == 0), stop=(k == K - 1),
                        )
                        nc.tensor.matmul(
                            psb, w_sb[64:128, 128 * k : 128 * (k + 1)],
                            sg[64:128, c0 : c0 + 512],
                            start=(k == 0), stop=(k == K - 1),
                        )
                    o0 = 256 * it
                    # ScalarE PReLU evicts both samples' PSUM banks in one
                    # 1024-wide activation (plain sequential write; strided
                    # ACT writes measure ~5x slower), then DVE pools each
                    # half via even/odd strided tensor_tensor (both read
                    # ports in parallel -> ~output-elems cycles)
                    sc = scpool.tile([128, 1024], mybir.dt.bfloat16, name="sc", tag="sc")
                    last_tile = t == NPAIR - 1 and it == NT - 1
                    if last_tile:
                        # parallel eviction of the final tile: half B pools
                        # on the DVE straight from PSUM while ACT prelus
                        # half A, so both flush chains finish ~together.
                        # Separate tiles: Tile serializes same-tile writers.
                        sc2 = consts.tile([128, 256], mybir.dt.bfloat16)
                        nc.vector.tensor_reduce(
                            sc2[:],
                            psb.rearrange("p (n two) -> p n two", two=2),
                            mybir.AxisListType.X,
                            ALU.max,
                        )
                        nc.scalar.activation(
                            sc[:, 0:512], psa, AF.Prelu, alpha=slope
                        )
                        scv = sc[:, 0:512].rearrange("p (j k) -> p j k", k=2)
                        nc.vector.tensor_tensor(
                            oba[:, o0 : o0 + 256],
                            scv[:, :, 0], scv[:, :, 1],
                            ALU.max,
                        )
                        nc.scalar.activation(
                            obb[:, o0 : o0 + 256], sc2[:],
                            AF.Prelu, alpha=slope,
                        )
                    else:
                        nc.scalar.activation(sc[:], ps2[:], AF.Prelu, alpha=slope)
                        for h, ob in ((0, oba), (1, obb)):
                            scv = sc[:, 512 * h : 512 * h + 512].rearrange(
                                "p (j k) -> p j k", k=2
                            )
                            nc.vector.tensor_tensor(
                                ob[:, o0 : o0 + 256],
                                scv[:, :, 0], scv[:, :, 1],
                                ALU.max,
                            )
                    # software-pipeline the NEXT pair's input + signs so the
                    # DVE queue has them before this pair's evictions finish
                    if t + 1 < NPAIR:
                        if it == 1:
                            start_pair(t + 1)
                        if 2 <= it < 2 + NCH:
                            emit_in_chunk(t + 1, it - 2)
                    # flush outputs every 2 tiles (per tile at the very end
                    # to shorten the DMA tail); half A rides the gpsimd DMA
                    # queue, half B the sync queue, so outputs never delay
                    # the next pair's input chunks
                    fin = t == NPAIR - 1 and it >= NT - 2
                    if it % 2 == 1 or fin:
                        s0 = 256 * it if fin else 256 * (it - 1)
                        sw = 256 if fin else 512
                        # only the very last tile's flushes ride the fast
                        # sync ring (tile 6's half A has slack; keeping it
                        # on gpsimd avoids serializing 4 issue slots on
                        # sync right at the end)
                        a_dma = (
                            nc.sync.dma_start
                            if fin and it == NT - 1
                            else nc.gpsimd.dma_start
                        )
                        a_dma(
                            oflat[128 * (2 * t) : 128 * (2 * t) + 128, s0 : s0 + sw],
                            oba[:, s0 : s0 + sw],
                        )
                        nc.sync.dma_start(
                            oflat[128 * (2 * t + 1) : 128 * (2 * t + 1) + 128, s0 : s0 + sw],
                            obb[:, s0 : s0 + sw],
                        )
    return nc


def _split_sync_waits_json(bir: bytes) -> bytes:
    """Walrus in this toolchain accepts at most one sync-wait per instruction.
    Hoist multi-wait sync_info lists into preceding single-wait EventSemaphore
    instructions on the same engine queue (the same form engine.wait_ge()
    lowers to), preserving program order and on_update placement."""
    j = json.loads(bir)
    n_split = 0
    for fn in j.get("functions", []):
        for blk in fn.get("blocks", []):
            ins_list = blk.get("instructions")
            if not ins_list:
                continue
            out = []
            for ins in ins_list:
                si = ins.get("sync_info")
                waits = si.get("on_wait") if si else None
                if waits and len(waits) > 1:
                    for i, w in enumerate(waits):
                        out.append(
                            {
                                "debug": ins.get("debug", 0),
                                "engine": ins["engine"],
                                "ins": [],
                                "outs": [],
                                "name": f"{ins['name']}-antw{i}",
                                "opcode": "EventSemaphore",
                                "sync_info": {"on_update": [], "on_wait": [w]},
                            }
                        )
                    si["on_wait"] = []
                    n_split += 1
                out.append(ins)
            blk["instructions"] = out
    return json.dumps(j).encode()


def get_program() -> "bass.Bass":
    if "nc" not in _CACHE:
        nc = build_program()
        orig = nc.to_json_bytes
        nc.to_json_bytes = lambda: _split_sync_waits_json(orig())
        _CACHE["nc"] = nc
    return _CACHE["nc"]


def prep_inputs(I, bn_gamma, bn_beta, bn_mean, bn_var, conv_w, alpha, prelu_w):
    """Host-side folding: BN -> per-channel sign threshold; alpha and the
    +-0.5 sign representation (factor 2) -> weights; per-k lhsT blocks
    duplicated into both PE array halves."""
    f32 = np.float32
    gamma = np.asarray(bn_gamma, f32)
    beta = np.asarray(bn_beta, f32)
    mean = np.asarray(bn_mean, f32)
    var = np.asarray(bn_var, f32)
    s = gamma / np.sqrt(var + f32(BN_EPS))        # [CIN]
    theta = mean - beta / s                        # sign(s*x+t) == x > theta

    w = 2.0 * np.asarray(conv_w, f32) * np.asarray(alpha, f32)[:, None, None]
    Wb = np.zeros((128, K * 128), np.float32)
    for k in range(K):
        Wb[0:64, 128 * k : 128 * k + 128] = w[:, :, k].T
        Wb[64:128, 128 * k : 128 * k + 128] = w[:, :, k].T
    Wb = Wb.astype(ml_dtypes.bfloat16)

    a = f32(np.asarray(prelu_w, f32).reshape(-1)[0])
    sbp = np.zeros((128, 2), f32)
    sbp[0:64, 0] = theta
    sbp[64:128, 0] = theta
    sbp[:, 1] = a
    return Wb, sbp


def kernel(I, bn_gamma, bn_beta, bn_mean, bn_var, conv_w, alpha, prelu_w):
    I = np.ascontiguousarray(np.asarray(I, np.float32))
    assert I.shape == (B, CIN, L), I.shape
    Wb, sbp = prep_inputs(I, bn_gamma, bn_beta, bn_mean, bn_var, conv_w, alpha, prelu_w)

    nc = get_program()
    in_maps = [
        {"I8": I[BPC * c : BPC * (c + 1)], "W": Wb, "SBp": sbp} for c in range(N_CORES)
    ]
    res = run_bass_kernel_spmd(nc, in_maps, core_ids=list(range(N_CORES)))
    out = np.concatenate(
        [np.asarray(res.results[c]["O8"]) for c in range(N_CORES)], axis=0
    )
    return np.ascontiguousarray(out.astype(np.float32))
